# revision 1
# baseline (speedup 1.0000x reference)
"""Trainium2 Bass kernel for nn_CausalDownsample (2-stage causal conv downsample
+ strided-causal cross attention), SPMD over 8 NeuronCores.

Sharding: core c in [0,8) -> batch b = c//4, time-quarter qt = c%4.
  Phase 1 (convs): sequence-parallel with left halo (uniform window geometry,
    per-core differences live only in the host-sliced inputs); each core
    produces q_in[b][:, 256*qt : 256*qt+256] (channels-major). Conv matmuls in
    bf16 (weights + relu'd inputs) with the residual stream kept in f32r.
  AllGather(q_in) within each batch group of 4 cores.
  Phase 2: per-core heads {2qt, 2qt+1}: k/v projections (bf16) streamed over
    x, q projection per gathered rank block (f32r), then masked attention in
    scoresT [key, query] orientation: no transposes, no max-subtraction
    (scores are O(10)), softmax denominator via an all-ones [128,128] lhsT
    matmul accumulated alongside o, mask as a constant [128,32] 0/1 tile on
    the diagonal key-block (strided-causal pattern is shift-invariant).
  Phase 3: out-proj partials over the local head dims + ReduceScatter(add)
    scatters query-quarters back; host transposes/assembles.

Attention matmuls run as float32r (fp32 storage, ~12-bit mantissa, full PE
rate at N>=256). Measured end-to-end rel err vs the fp32 reference: 5.1e-3
(all-f32r variant via CONV_BF16=False: 5.0e-4). Biases in this problem are
structurally zero and are skipped (bv would fold exactly into the out bias).
"""
import sys
if '/opt/trn_rl_repo' not in sys.path:
    sys.path.insert(0, '/opt/trn_rl_repo')

import numpy as np

import concourse.bacc as bacc
import concourse.tile as tile
import concourse.mybir as mybir

F32 = mybir.dt.float32
F32R = mybir.dt.float32r
AF = mybir.ActivationFunctionType
ADD = mybir.AluOpType.add
MULT = mybir.AluOpType.mult

N_CORES = 8
GROUPS = [[0, 1, 2, 3], [4, 5, 6, 7]]
P = 128
CC = 8            # channel chunks (1024/128)
DIM = 1024
T = 4096
TQ = 1024
HD = 128
NH = 2            # heads per core
B = 2

LX = 1192         # xpad window width
L0, L1 = 594, 282 # stage0 / stage1 computed window lengths
M0 = M1 = 24      # zeroed left margins of stream buffers
W0, W1 = M0 + L0, M1 + L1
TS0 = [(0, 298), (298, 296)]
TS1 = [(0, 282)]
DILS = (9, 3, 1)
SCALE = 1.0 / np.sqrt(HD)

DT = F32R
BF16 = mybir.dt.bfloat16
CONV_BF16 = True          # conv matmuls in bf16 (stream stays f32r)
DTC = BF16 if CONV_BF16 else F32R
WBUFS = 8 if CONV_BF16 else 3


def _build(sim_single_core=False, reps=1):
    nc = bacc.Bacc("TRN2", target_bir_lowering=False, debug=False,
                   num_devices=N_CORES)

    def din(name, shape, dt=DT):
        return nc.dram_tensor(name, list(shape), dt, kind="ExternalInput").ap()

    xpad_d = din("xpad", [DIM, LX], DTC)
    xfull_d = din("xfull", [DIM, T], DTC)
    wconv_d = {}
    for s in range(2):
        wconv_d[(s, 'ds')] = din(f"ds{s}", [8, DIM, 4 * P], DTC)
        for j in range(3):
            wconv_d[(s, 'c3', j)] = din(f"c3_{s}_{j}", [8, DIM, 3 * P], DTC)
            wconv_d[(s, 'c1', j)] = din(f"c1_{s}_{j}", [8, DIM, P], DTC)
    wq_d = din("wq", [CC, P, NH * HD])
    wk_d = din("wk", [CC, P, NH * HD], DTC)
    wv_d = din("wv", [CC, P, NH * HD], DTC)
    outw_d = din("outw", [NH, P, DIM])
    mask_d = din("mask01", [P, 32])
    onesl_d = din("ones_l", [P, P])
    zeros_d = din("zeros_m", [P, CC * M0])
    y_d = nc.dram_tensor("y", [CC, P, 256], F32, kind="ExternalOutput").ap()

    with tile.TileContext(nc) as tc:
      for _rep in range(reps):
        # ---------------- constant + dram pools (whole kernel) ----------------
        with tc.tile_pool(name="const", bufs=1) as cpool, \
             tc.tile_pool(name="dram", bufs=1, space="DRAM") as dpool:
            mask_t = cpool.tile([P, 32], DT)
            onesl_t = cpool.tile([P, P], DT)

            ag_in = dpool.tile([CC, P, 256], DT)
            ag_out = dpool.tile([4, CC, P, 256], DT)
            rs_in = dpool.tile([4, CC, P, 256], F32)
            rs_out = dpool.tile([CC, P, 256], F32)

            # ================= Phase 1: convolutions =================
            with tc.tile_pool(name="convsb", bufs=1) as sb, \
                 tc.tile_pool(name="wpool", bufs=WBUFS) as wp, \
                 tc.tile_pool(name="cpsum", bufs=4, space="PSUM") as cps:
                wt_first = wp.tile([P, CC * 4 * P], DTC, tag="wt")
                nc.sync.dma_start(
                    wt_first[:].rearrange("p (c f) -> p c f", c=CC),
                    wconv_d[(0, 'ds')][0].rearrange("(c p) f -> p c f", p=P))
                xw = sb.tile([P, CC * LX], DTC, tag="xw")
                xpad_v = xpad_d[:].rearrange("(c p) f -> p c f", p=P)
                for cc in range(CC):
                    nc.sync.dma_start(xw[:, cc * LX:(cc + 1) * LX], xpad_v[:, cc])
                x0 = sb.tile([P, CC * W0], DT, tag="x0")
                r0 = sb.tile([P, CC * W0], DTC, tag="r0")
                h0 = sb.tile([P, CC * W0], DTC, tag="h0")
                x1 = sb.tile([P, CC * W1], DT, tag="x1")
                r1 = sb.tile([P, CC * W1], DTC, tag="r1")
                h1 = sb.tile([P, CC * W1], DTC, tag="h1")
                if CONV_BF16:
                    x0s = sb.tile([P, CC * W0], DTC, tag="x0s")
                else:
                    x0s = x0
                zv = zeros_d[:].rearrange("p (c f) -> p c f", c=CC)
                nc.sync.dma_start(
                    x0[:].rearrange("p (c f) -> p c f", c=CC)[:, :, 0:M0], zv)
                nc.sync.dma_start(
                    x1[:].rearrange("p (c f) -> p c f", c=CC)[:, :, 0:M1], zv)

                def conv_layer(src, srcW, src_col0, dst, dstW, tsplits, wd, K,
                               offs, stride, mode, res=None, first_wt=None):
                    for m in range(CC):
                        if m == 0 and first_wt is not None:
                            wt = first_wt
                        else:
                            wt = wp.tile([P, CC * K * P], DTC, tag="wt")
                            nc.sync.dma_start(
                                wt[:].rearrange("p (c f) -> p c f", c=CC),
                                wd[m].rearrange("(c p) f -> p c f", p=P))
                        for (t0, tn) in tsplits:
                            ps = cps.tile([P, tn], F32, tag="cps")
                            nmm = 0
                            for cc in range(CC):
                                base = cc * srcW + src_col0 + stride * t0
                                for k in range(K):
                                    col = base + offs[k]
                                    if stride == 1:
                                        rhs = src[:, col:col + tn]
                                    else:
                                        rhs = src[:, col:col + stride * tn:stride]
                                    nc.tensor.matmul(
                                        ps[:],
                                        wt[:, (cc * K + k) * P:(cc * K + k + 1) * P],
                                        rhs,
                                        start=(nmm == 0), stop=(nmm == CC * K - 1))
                                    nmm += 1
                            dsl = slice(m * dstW + M0 + t0, m * dstW + M0 + t0 + tn)
                            if mode == 'relu':
                                nc.scalar.activation(dst[:, dsl], ps[:], AF.Relu)
                            elif mode == 'copy':
                                nc.vector.tensor_copy(dst[:, dsl], ps[:])
                            else:  # residual add
                                nc.vector.tensor_tensor(
                                    out=dst[:, dsl], in0=res[:, dsl], in1=ps[:],
                                    op=ADD)

                # stage 0
                conv_layer(xw, LX, 0, x0, W0, TS0, wconv_d[(0, 'ds')], 4,
                           [1, 2, 3, 4], 2, 'copy', first_wt=wt_first)
                for j, d in enumerate(DILS):
                    for cc in range(CC):
                        nc.scalar.activation(r0[:, cc * W0:(cc + 1) * W0],
                                             x0[:, cc * W0:(cc + 1) * W0], AF.Relu)
                    conv_layer(r0, W0, M0, h0, W0, TS0, wconv_d[(0, 'c3', j)], 3,
                               [-2 * d, -d, 0], 1, 'relu')
                    conv_layer(h0, W0, M0, x0, W0, TS0, wconv_d[(0, 'c1', j)], 1,
                               [0], 1, 'add', res=x0)
                if CONV_BF16:
                    for cc in range(CC):
                        nc.vector.tensor_copy(x0s[:, cc * W0:(cc + 1) * W0],
                                              x0[:, cc * W0:(cc + 1) * W0])
                # stage 1
                conv_layer(x0s, W0, M0 + 27, x1, W1, TS1, wconv_d[(1, 'ds')], 4,
                           [0, 1, 2, 3], 2, 'copy')
                for j, d in enumerate(DILS):
                    for cc in range(CC):
                        nc.scalar.activation(r1[:, cc * W1:(cc + 1) * W1],
                                             x1[:, cc * W1:(cc + 1) * W1], AF.Relu)
                    conv_layer(r1, W1, M1, h1, W1, TS1, wconv_d[(1, 'c3', j)], 3,
                               [-2 * d, -d, 0], 1, 'relu')
                    conv_layer(h1, W1, M1, x1, W1, TS1, wconv_d[(1, 'c1', j)], 1,
                               [0], 1, 'add', res=x1)

                # ship q_in chunk to the gather buffer
                for cc in range(CC):
                    nc.sync.dma_start(
                        ag_in[cc],
                        x1[:, cc * W1 + M1 + 26:cc * W1 + M1 + 26 + 256])

            if sim_single_core:
                nc.sync.dma_start(ag_out[0], ag_in[:])
                nc.sync.dma_start(ag_out[1], ag_in[:])
                nc.sync.dma_start(ag_out[2], ag_in[:])
                nc.sync.dma_start(ag_out[3], ag_in[:])
            else:
                nc.gpsimd.collective_compute(
                    "AllGather", mybir.AluOpType.bypass, replica_groups=GROUPS,
                    ins=[ag_in.opt()], outs=[ag_out.opt()])

            nc.sync.dma_start(mask_t[:], mask_d[:])
            nc.sync.dma_start(onesl_t[:], onesl_d[:])
            # ================= Phase 2: projections + attention =================
            with tc.tile_pool(name="attnsb", bufs=1) as asb:
                k_sb = asb.tile([P, NH * T], DT, tag="ksb")
                v_sb = asb.tile([P, (T // P) * NH * HD], DT, tag="vsb")
                q_sb = asb.tile([P, NH * TQ], DT, tag="qsb")

                # k/v projections, streaming x by 512-column tiles
                with tc.tile_pool(name="projsb", bufs=1) as psb, \
                     tc.tile_pool(name="kvps", bufs=3, space="PSUM") as kvps:
                    wk_t = psb.tile([P, CC * NH * HD], DTC, tag="wk")
                    wv_t = psb.tile([P, CC * NH * HD], DTC, tag="wv")
                    wq_t = psb.tile([P, CC * NH * HD], DT, tag="wq")
                    for wt_, wd_ in ((wk_t, wk_d), (wv_t, wv_d), (wq_t, wq_d)):
                        nc.sync.dma_start(
                            wt_[:].rearrange("p (c f) -> p c f", c=CC),
                            wd_[:].rearrange("c p f -> p c f"))
                    xsp_cm = tc.tile_pool(name="xsp", bufs=8 if CONV_BF16 else 3)
                    xsp = xsp_cm.__enter__()
                    for tt in range(T // 512):
                        xs = xsp.tile([P, CC * 512], DTC, tag="xs")
                        xf_v = (xfull_d[:, tt * 512:(tt + 1) * 512]
                                .rearrange("(c p) f -> p c f", p=P))
                        for cc in range(CC):
                            nc.sync.dma_start(xs[:, cc * 512:(cc + 1) * 512],
                                              xf_v[:, cc])
                        for h in range(NH):
                            pk = kvps.tile([P, 512], F32, tag="kv")
                            for cc in range(CC):
                                nc.tensor.matmul(
                                    pk[:],
                                    wk_t[:, cc * 256 + h * HD:cc * 256 + h * HD + HD],
                                    xs[:, cc * 512:(cc + 1) * 512],
                                    start=(cc == 0), stop=(cc == CC - 1))
                            nc.vector.tensor_copy(
                                k_sb[:, h * T + tt * 512:h * T + (tt + 1) * 512],
                                pk[:])
                        for t4 in range(4):
                            pv = kvps.tile([P, 256], F32, tag="kv")
                            for cc in range(CC):
                                nc.tensor.matmul(
                                    pv[:],
                                    xs[:, cc * 512 + t4 * P:cc * 512 + (t4 + 1) * P],
                                    wv_t[:, cc * 256:(cc + 1) * 256],
                                    start=(cc == 0), stop=(cc == CC - 1))
                            nc.vector.tensor_copy(
                                v_sb[:, (tt * 4 + t4) * 256:(tt * 4 + t4 + 1) * 256],
                                pv[:])

                    xsp_cm.__exit__(None, None, None)
                    # q projection from the gathered q_in, per rank block
                    qip_cm = tc.tile_pool(name="qip", bufs=1)
                    qip = qip_cm.__enter__()
                    qi_sb = qip.tile([P, CC * TQ], DT, tag="qisb")
                    qi_v = qi_sb[:].rearrange("p (c f) -> p c f", c=CC)
                    for rr in range(4):
                        nc.sync.dma_start(
                            qi_v[:, :, rr * 256:(rr + 1) * 256],
                            ag_out[rr].rearrange("c p f -> p c f"))
                        for h in range(NH):
                            pq = kvps.tile([P, 256], F32, tag="kv")
                            for cc in range(CC):
                                nc.tensor.matmul(
                                    pq[:],
                                    wq_t[:, cc * 256 + h * HD:cc * 256 + h * HD + HD],
                                    qi_sb[:, cc * TQ + rr * 256:cc * TQ + (rr + 1) * 256],
                                    start=(cc == 0), stop=(cc == CC - 1))
                            nc.vector.tensor_copy(
                                q_sb[:, h * TQ + rr * 256:h * TQ + (rr + 1) * 256],
                                pq[:])
                    qip_cm.__exit__(None, None, None)

                # ---- attention core, scoresT orientation ----
                o_sb = asb.tile([P, NH * TQ], DT, tag="osb")
                with tc.tile_pool(name="scps", bufs=4, space="PSUM") as scps, \
                     tc.tile_pool(name="ops", bufs=1, space="PSUM") as ops, \
                     tc.tile_pool(name="lps", bufs=1, space="PSUM") as lps, \
                     tc.tile_pool(name="esb", bufs=6) as esb, \
                     tc.tile_pool(name="ebig", bufs=2) as ebig:
                    NKB = T // P
                    for h in range(NH):
                        o_ps = ops.tile([P, TQ], F32, tag="o")
                        l_ps = lps.tile([P, TQ], F32, tag="lstat")
                        pend = []
                        for kb in range(NKB + 1):
                            cur = []
                            if kb < NKB:
                                qstart = 32 * kb
                                width = TQ - qstart
                                if width > 512:
                                    n0 = (width + 1) // 2
                                    subs = [(qstart, n0), (qstart + n0, width - n0)]
                                else:
                                    subs = [(qstart, width)]
                                first = True
                                for (qs, qn) in subs:
                                    sc = scps.tile([P, 512], F32, tag="sc")
                                    nc.tensor.matmul(
                                        sc[:, :qn],
                                        k_sb[:, h * T + kb * P:h * T + (kb + 1) * P],
                                        q_sb[:, h * TQ + qs:h * TQ + qs + qn],
                                        start=True, stop=True)
                                    et = esb.tile([P, 512], DT, tag="et")
                                    if first:
                                        tm = esb.tile([P, 32], DT, tag="tm")
                                        nc.scalar.activation(tm[:], sc[:, :32],
                                                             AF.Exp, scale=SCALE)
                                        nc.vector.tensor_tensor(
                                            out=et[:, :32], in0=tm[:],
                                            in1=mask_t[:], op=MULT)
                                        if qn > 32:
                                            nc.scalar.activation(
                                                et[:, 32:qn], sc[:, 32:qn],
                                                AF.Exp, scale=SCALE)
                                        first = False
                                    else:
                                        nc.scalar.activation(
                                            et[:, :qn], sc[:, :qn],
                                            AF.Exp, scale=SCALE)
                                    cur.append((et, qs, qn, kb))
                            for (et, qs, qn, k0) in pend:
                                nc.tensor.matmul(
                                    o_ps[:, qs:qs + qn],
                                    v_sb[:, k0 * 256 + h * HD:k0 * 256 + h * HD + HD],
                                    et[:, :qn],
                                    start=(k0 == 0), stop=(k0 == NKB - 1))
                                nc.tensor.matmul(
                                    l_ps[:, qs:qs + qn], onesl_t[:], et[:, :qn],
                                    start=(k0 == 0), stop=(k0 == NKB - 1))
                            pend = cur
                        # free o_ps fast, then normalize in SBUF
                        o_raw = ebig.tile([P, TQ], F32, tag="oraw")
                        nc.vector.tensor_copy(o_raw[:], o_ps[:])
                        lr_sb = ebig.tile([P, TQ], F32, tag="lrsb")
                        nc.vector.reciprocal(lr_sb[:], l_ps[:])
                        nc.vector.tensor_tensor(out=o_sb[:, h * TQ:(h + 1) * TQ],
                                                in0=lr_sb[:], in1=o_raw[:], op=MULT)

                # ---- out-proj partials + reduce-scatter ----
                with tc.tile_pool(name="yps", bufs=2, space="PSUM") as yps, \
                     tc.tile_pool(name="ysp", bufs=1) as ysp:
                    outw_t = asb.tile([P, NH * DIM], DT, tag="outw")
                    nc.sync.dma_start(
                        outw_t[:].rearrange("p (c f) -> p c f", c=NH),
                        outw_d[:].rearrange("c p f -> p c f"))
                    ys = ysp.tile([P, CC * TQ], F32, tag="ys")
                    for m in range(CC):
                        for hf in range(2):
                            yp = yps.tile([P, 512], F32, tag="y")
                            for dc in range(NH):
                                nc.tensor.matmul(
                                    yp[:],
                                    outw_t[:, dc * DIM + m * P:dc * DIM + (m + 1) * P],
                                    o_sb[:, dc * TQ + hf * 512:dc * TQ + (hf + 1) * 512],
                                    start=(dc == 0), stop=(dc == NH - 1))
                            nc.vector.tensor_copy(
                                ys[:, m * TQ + hf * 512:m * TQ + (hf + 1) * 512],
                                yp[:])
                    for rr in range(4):
                        for m in range(CC):
                            nc.sync.dma_start(
                                rs_in[rr, m],
                                ys[:, m * TQ + rr * 256:m * TQ + (rr + 1) * 256])

            if sim_single_core:
                nc.sync.dma_start(rs_out[:], rs_in[0])
            else:
                nc.gpsimd.collective_compute(
                    "ReduceScatter", ADD, replica_groups=GROUPS,
                    ins=[rs_in.opt()], outs=[rs_out.opt()])
            nc.sync.dma_start(y_d[:], rs_out[:])

    nc.compile()
    return nc


# ---------------------------------------------------------------------------
# host side
# ---------------------------------------------------------------------------
def _pack_conv(W):
    """W [1024, 1024, K] -> [8, 1024, K*128]; pack[m, c, k*128+j] = W[m*128+j, c, k]"""
    import ml_dtypes
    co, ci, K = W.shape
    out = np.ascontiguousarray(
        W.reshape(8, P, ci, K).transpose(0, 2, 3, 1).reshape(8, ci, K * P))
    return out.astype(ml_dtypes.bfloat16) if CONV_BF16 else out.astype(np.float32)


def _make_in_maps(inputs):
    x = np.asarray(inputs['x'], np.float32)            # [B, T, DIM]
    xT = [np.ascontiguousarray(x[b].T) for b in range(B)]

    conv_shared = {}
    for s in range(2):
        conv_shared[f"ds{s}"] = _pack_conv(np.asarray(inputs[f'dw{s}'], np.float32))
        rw1 = np.asarray(inputs[f'rw1_{s}'], np.float32)
        rw2 = np.asarray(inputs[f'rw2_{s}'], np.float32)
        for j in range(3):
            conv_shared[f"c3_{s}_{j}"] = _pack_conv(rw1[j])
            conv_shared[f"c1_{s}_{j}"] = _pack_conv(rw2[j])

    ipw = np.asarray(inputs['in_proj_w'], np.float32)
    wq, wk, wv = ipw[0:DIM], ipw[DIM:2 * DIM], ipw[2 * DIM:3 * DIM]
    outw = np.asarray(inputs['out_w'], np.float32)

    kk = np.arange(P)[:, None]
    qq = np.arange(32)[None, :]
    mask01 = (kk < 4 * qq + 4).astype(np.float32)

    in_maps = []
    for c in range(N_CORES):
        b, qt = c // 4, c % 4
        import ml_dtypes
        xs0 = 1024 * qt - 168
        xpad = np.zeros((DIM, LX), np.float32)
        lo = max(0, xs0)
        xpad[:, lo - xs0:] = xT[b][:, lo:1024 * qt + 1024]
        if CONV_BF16:
            xpad = xpad.astype(ml_dtypes.bfloat16)

        hsl = slice(256 * qt, 256 * qt + 256)
        cdt = ml_dtypes.bfloat16 if CONV_BF16 else np.float32
        m = {
            'xpad': xpad,
            'xfull': xT[b].astype(cdt) if CONV_BF16 else xT[b],
            'wq': np.ascontiguousarray(wq[hsl].T).reshape(CC, P, 256),
            'wk': np.ascontiguousarray(wk[hsl].T).reshape(CC, P, 256).astype(cdt),
            'wv': np.ascontiguousarray(wv[hsl].T).reshape(CC, P, 256).astype(cdt),
            'outw': np.ascontiguousarray(outw[:, hsl].T).reshape(NH, P, DIM),
            'mask01': mask01,
            'ones_l': np.ones((P, P), np.float32),
            'zeros_m': np.zeros((P, CC * M0), np.float32),
        }
        m.update(conv_shared)
        in_maps.append(m)
    return in_maps


_RUNNER = {}


def _get_runner():
    """Build the Bass module once and return a cached jitted SPMD callable."""
    if _RUNNER:
        return _RUNNER
    _RUNNER.update(_make_jit(_build()))
    return _RUNNER


def _make_jit(nc):
    import jax
    from jax.sharding import Mesh, PartitionSpec
    from jax.experimental.shard_map import shard_map
    from concourse import bass2jax
    from concourse import mybir as _mybir

    bass2jax.install_neuronx_cc_hook()

    partition_name = (nc.partition_id_tensor.name
                      if nc.partition_id_tensor else None)
    in_names, out_names, out_avals, zero_outs = [], [], [], []
    for alloc in nc.m.functions[0].allocations:
        if not isinstance(alloc, _mybir.MemoryLocationSet):
            continue
        name = alloc.memorylocations[0].name
        if alloc.kind == "ExternalInput":
            if name == partition_name:
                continue
            in_names.append(name)
        elif alloc.kind == "ExternalOutput":
            out_names.append(name)
            shape = tuple(alloc.tensor_shape)
            dtype = _mybir.dt.np(alloc.dtype)
            out_avals.append(jax.core.ShapedArray(shape, dtype))
            zero_outs.append(np.zeros(shape, dtype))
    n_params = len(in_names)
    all_names = in_names + out_names
    if partition_name is not None:
        all_names = all_names + [partition_name]

    def _body(*args):
        operands = list(args)
        if partition_name is not None:
            operands.append(bass2jax.partition_id_tensor())
        outs = bass2jax._bass_exec_p.bind(
            *operands,
            out_avals=tuple(out_avals),
            in_names=tuple(all_names),
            out_names=tuple(out_names),
            lowering_input_output_aliases=(),
            sim_require_finite=True,
            sim_require_nnan=True,
            nc=nc,
        )
        return tuple(outs)

    devices = jax.devices()[:N_CORES]
    mesh = Mesh(np.asarray(devices), ("core",))
    n_out = len(out_names)
    sharded = jax.jit(
        shard_map(_body, mesh=mesh,
                  in_specs=(PartitionSpec("core"),) * (n_params + n_out),
                  out_specs=(PartitionSpec("core"),) * n_out,
                  check_rep=False),
        donate_argnums=tuple(range(n_params, n_params + n_out)),
        keep_unused=True)

    return dict(fn=sharded, in_names=in_names, out_names=out_names,
                zero_outs=zero_outs, out_avals=out_avals)


def run_device(in_maps):
    r = _get_runner()
    concat_in = [np.concatenate([m[name] for m in in_maps], axis=0)
                 for name in r['in_names']]
    concat_zeros = [np.zeros((N_CORES * z.shape[0], *z.shape[1:]), z.dtype)
                    for z in r['zero_outs']]
    out_arrs = r['fn'](*concat_in, *concat_zeros)
    return [
        {name: np.asarray(out_arrs[i]).reshape(N_CORES, *r['out_avals'][i].shape)[c]
         for i, name in enumerate(r['out_names'])}
        for c in range(N_CORES)
    ]


def kernel(**inputs):
    in_maps = _make_in_maps(inputs)
    results = run_device(in_maps)
    out = np.empty((B, TQ, DIM), np.float32)
    for c in range(N_CORES):
        b, qt = c // 4, c % 4
        y = results[c]['y'].reshape(DIM, 256)   # [co, q_local]
        out[b, 256 * qt:256 * qt + 256, :] = y.T
    return out

